# revision 18
# baseline (speedup 1.0000x reference)
"""FSQ codebook kernel for Trainium2 (8 NeuronCores, data-parallel over tokens).

Computes, for x:(8,8192,1280) f32, W:(8,1280) f32, b:(8,) f32:
    h  = x.reshape(-1,1280) @ W.T + b            # (65536, 8)
    mu = sum_k 3^k * (1 + round(tanh(h)*SCALE))  # base-3 code, int32
    -> (8, 8192) int32

The tanh/round/scale pipeline is replaced by an exact fp32 threshold:
    round(tanh(h)*SCALE) = +1  iff  h >= T_POS
                         = -1  iff  h <= -T_POS      (bit-exact, verified)
so digit value (1+r) = [h >= T_POS] + [h > -T_POS] and
    mu = sum_k 3^k*[h_k >= T] + sum_k 3^k*[h_k > -T].

fp16x2 path (default): x and W are scaled by 2^10 and Dekker-split on the
host into hi/lo fp16 pairs (x*2^10 = hi + lo + O(2^-24); same total bytes).
The 16-bit XBAR DMA-transpose loads x already transposed (d on partitions),
eliminating all PE transposes and PSUM->SBUF copies. The GEMM is 4 fp16
matmul products per d-tile at full PE rate, accumulating the 2^20-scaled h
in fp32 PSUM; thresholds are scaled by exactly 2^20 (power-of-two => the
comparison is unchanged).

fp32 path (fallback): PE-transpose 128x128 blocks + ACT/DVE copies + fp32
matmuls (weight stationary).
"""

import numpy as np

# exact fp32 threshold: minimal fp32 v with round(tanh(v)*SCALE) == 1
T_POS = float(np.uint32(0x3F0CCB15).view(np.float32))
SPLIT_SCALE = 1024.0  # 2^10 per operand; h is scaled by 2^20

N_CORES = 8
TOK_PER_CORE = 8192
D = 1280
K = 8
D_TILES = D // 128            # 10

MODE = "fp16x2t2"
N_PRODUCTS = 4                # hi*hi, hi*lo, lo*hi, lo*lo

# fp32-path tiling
SUPER = 4
TOKS = SUPER * 128            # 512
N_SUPER = TOK_PER_CORE // TOKS

# fp16x2-path tiling: 1024-token groups, matmul N=512 halves
GTOK = 1024
N_GROUP = TOK_PER_CORE // GTOK  # 8

_cached = {}


def _build_fp16x2(repeat=1):
    from contextlib import ExitStack

    from concourse import bacc, mybir, tile

    f16 = mybir.dt.float16
    f32 = mybir.dt.float32
    i32 = mybir.dt.int32

    nc = bacc.Bacc("TRN2", target_bir_lowering=False, debug=False)

    xhi_ap = nc.dram_tensor("xhi", [TOK_PER_CORE, D], f16, kind="ExternalInput").ap()
    xlo_ap = nc.dram_tensor("xlo", [TOK_PER_CORE, D], f16, kind="ExternalInput").ap()
    wthi_ap = nc.dram_tensor("wthi", [D, K], f16, kind="ExternalInput").ap()
    wtlo_ap = nc.dram_tensor("wtlo", [D, K], f16, kind="ExternalInput").ap()
    b_ap = nc.dram_tensor("bias", [1, K], f32, kind="ExternalInput").ap()
    pw_ap = nc.dram_tensor("powers", [K, 1], f32, kind="ExternalInput").ap()
    out_ap = nc.dram_tensor(
        "out", [1, TOK_PER_CORE], i32, kind="ExternalOutput"
    ).ap()

    T_HI = T_POS * SPLIT_SCALE * SPLIT_SCALE

    with tile.TileContext(nc) as tc, ExitStack() as ctx:
        const_pool = ctx.enter_context(tc.tile_pool(name="const", bufs=1))
        xt_pool = ctx.enter_context(tc.tile_pool(name="xt", bufs=3))
        val_pool = ctx.enter_context(tc.tile_pool(name="val", bufs=3))
        mu_pool = ctx.enter_context(tc.tile_pool(name="mu", bufs=1))
        ps_h = ctx.enter_context(tc.tile_pool(name="ps_h", bufs=4, space="PSUM"))
        ps_m = ctx.enter_context(tc.tile_pool(name="ps_m", bufs=2, space="PSUM"))

        # stacked stationary, 40 cols per d-tile: cols [0:8]=Whi_dt,
        # [32:40]=Wlo_dt (partition windows must start at multiples of 32;
        # the unused middle columns cost nothing — matmul time is N-bound)
        WP = 40
        wpair_sb = const_pool.tile([128, D_TILES * WP], f16)
        nc.vector.memset(wpair_sb[:], 0)
        nc.sync.dma_start(
            wpair_sb[:].rearrange("p (dt c) -> p dt c", dt=D_TILES)[:, :, 0:K],
            wthi_ap.rearrange("(dt p) k -> p dt k", p=128),
        )
        nc.sync.dma_start(
            wpair_sb[:].rearrange("p (dt c) -> p dt c", dt=D_TILES)[
                :, :, 32 : 32 + K
            ],
            wtlo_ap.rearrange("(dt p) k -> p dt k", p=128),
        )
        b_sb = const_pool.tile([1, K], f32)
        nc.sync.dma_start(b_sb[:], b_ap[:])
        pw_sb = const_pool.tile([K, 1], f32)
        nc.sync.dma_start(pw_sb[:], pw_ap[:])
        ones_row = const_pool.tile([1, 512], f32)
        nc.vector.memset(ones_row[:], 1.0)

        mu_i32 = mu_pool.tile([1, TOK_PER_CORE], i32)

        for _rep in range(repeat):
            for g in range(N_GROUP):
                t0 = g * GTOK
                # DMA-transpose loads: xthi[d, dt*GTOK + t] = xhi[t0+t, dt*128+d]
                xthi = xt_pool.tile([128, D_TILES * GTOK], f16, name="xthi")
                xtlo = xt_pool.tile([128, D_TILES * GTOK], f16, name="xtlo")
                for dt in range(D_TILES):
                    nc.sync.dma_start(
                        xthi[:, dt * GTOK : (dt + 1) * GTOK],
                        xhi_ap[t0 : t0 + GTOK, dt * 128 : (dt + 1) * 128],
                        transpose=True,
                    )
                    nc.sync.dma_start(
                        xtlo[:, dt * GTOK : (dt + 1) * GTOK],
                        xlo_ap[t0 : t0 + GTOK, dt * 128 : (dt + 1) * 128],
                        transpose=True,
                    )
                for half in range(2):
                    sl = lambda dt: slice(
                        dt * GTOK + half * 512, dt * GTOK + half * 512 + 512
                    )
                    # h40 rows 0-7 += Whi^T@(xthi+xtlo); rows 32-39 += Wlo^T@(...)
                    # all 4 Dekker products in 2 matmuls per d-tile
                    h40 = ps_h.tile([WP, 512], f32)
                    first = True
                    for dt in range(D_TILES):
                        for xsb in (xthi, xtlo):
                            nc.tensor.matmul(
                                h40[:],
                                lhsT=wpair_sb[:, dt * WP : (dt + 1) * WP],
                                rhs=xsb[:, sl(dt)],
                                start=first,
                                stop=False,
                            )
                            first = False
                    nc.tensor.matmul(
                        h40[0:K, :],
                        lhsT=b_sb[:],
                        rhs=ones_row[:],
                        start=False,
                        stop=True,
                    )

                    # h = rows[0:8] + rows[32:40]; val = [h >= T] + [h > -T]
                    # (tensor_tensor may read only one PSUM operand)
                    hlo_sb = val_pool.tile([K, 512], f32, name="hlo_sb")
                    nc.vector.tensor_copy(hlo_sb[:], h40[32 : 32 + K, :])
                    hsum = val_pool.tile([K, 512], f32, name="hsum")
                    nc.vector.tensor_add(hsum[:], h40[0:K, :], hlo_sb[:])
                    val1 = val_pool.tile([K, 512], f32, name="val1")
                    nc.vector.tensor_scalar(
                        out=val1[:],
                        in0=hsum[:],
                        scalar1=T_HI,
                        scalar2=None,
                        op0=mybir.AluOpType.is_ge,
                    )
                    val = val_pool.tile([K, 512], f32, name="val")
                    nc.vector.scalar_tensor_tensor(
                        out=val[:],
                        in0=hsum[:],
                        scalar=-T_HI,
                        in1=val1[:],
                        op0=mybir.AluOpType.is_gt,
                        op1=mybir.AluOpType.add,
                    )
                    # mu = powers^T @ val   (K=8 contraction)
                    mu_ps = ps_m.tile([1, 512], f32)
                    nc.tensor.matmul(
                        mu_ps[:], lhsT=pw_sb[:], rhs=val[:], start=True, stop=True
                    )
                    base = t0 + half * 512
                    nc.vector.tensor_copy(
                        mu_i32[:, base : base + 512], mu_ps[:]
                    )

        nc.sync.dma_start(out_ap[:], mu_i32[:])

    nc.compile()
    return nc


def _build_fp16x2t(repeat=1):
    """Host-pretransposed fp16x2: x is uploaded as one [128, G*DT*2*GTOK]
    fp16 tensor laid out [p, g, dt, hl, t] so each 1024-token group is a
    single fully-contiguous 5.2MB DMA (40KB/partition row).  No XBAR
    transpose, no per-tile DMA — the load runs at full HBM bandwidth and
    fans out across all 16 HW DGE engines automatically."""
    from contextlib import ExitStack

    from concourse import bacc, mybir, tile

    f16 = mybir.dt.float16
    f32 = mybir.dt.float32
    i32 = mybir.dt.int32

    nc = bacc.Bacc("TRN2", target_bir_lowering=False, debug=False)

    GFREE = D_TILES * 2 * GTOK          # 20480 per group
    xt_ap = nc.dram_tensor(
        "xt", [128, N_GROUP * GFREE], f16, kind="ExternalInput"
    ).ap()
    wthi_ap = nc.dram_tensor("wthi", [D, K], f16, kind="ExternalInput").ap()
    wtlo_ap = nc.dram_tensor("wtlo", [D, K], f16, kind="ExternalInput").ap()
    b_ap = nc.dram_tensor("bias", [1, K], f32, kind="ExternalInput").ap()
    pw_ap = nc.dram_tensor("powers", [K, 1], f32, kind="ExternalInput").ap()
    out_ap = nc.dram_tensor(
        "out", [1, TOK_PER_CORE], i32, kind="ExternalOutput"
    ).ap()

    T_HI = T_POS * SPLIT_SCALE * SPLIT_SCALE

    with tile.TileContext(nc) as tc, ExitStack() as ctx:
        const_pool = ctx.enter_context(tc.tile_pool(name="const", bufs=1))
        xt_pool = ctx.enter_context(tc.tile_pool(name="xt", bufs=3))
        val_pool = ctx.enter_context(tc.tile_pool(name="val", bufs=3))
        mu_pool = ctx.enter_context(tc.tile_pool(name="mu", bufs=1))
        ps_h = ctx.enter_context(tc.tile_pool(name="ps_h", bufs=4, space="PSUM"))
        ps_m = ctx.enter_context(tc.tile_pool(name="ps_m", bufs=2, space="PSUM"))

        # stacked stationary, 40 cols per d-tile: cols [0:8]=Whi_dt,
        # [32:40]=Wlo_dt (partition windows must start at multiples of 32)
        WP = 40
        wpair_sb = const_pool.tile([128, D_TILES * WP], f16)
        nc.vector.memset(wpair_sb[:], 0)
        nc.sync.dma_start(
            wpair_sb[:].rearrange("p (dt c) -> p dt c", dt=D_TILES)[:, :, 0:K],
            wthi_ap.rearrange("(dt p) k -> p dt k", p=128),
        )
        nc.sync.dma_start(
            wpair_sb[:].rearrange("p (dt c) -> p dt c", dt=D_TILES)[
                :, :, 32 : 32 + K
            ],
            wtlo_ap.rearrange("(dt p) k -> p dt k", p=128),
        )
        b_sb = const_pool.tile([1, K], f32)
        nc.sync.dma_start(b_sb[:], b_ap[:])
        pw_sb = const_pool.tile([K, 1], f32)
        nc.sync.dma_start(pw_sb[:], pw_ap[:])
        ones_row = const_pool.tile([1, 512], f32)
        nc.vector.memset(ones_row[:], 1.0)

        mu_i32 = mu_pool.tile([1, TOK_PER_CORE], i32)

        for _rep in range(repeat):
            for g in range(N_GROUP):
                xt = xt_pool.tile([128, GFREE], f16, name="xt")
                nc.sync.dma_start(
                    xt[:], xt_ap[:, g * GFREE : (g + 1) * GFREE]
                )
                for half in range(2):
                    # h40 rows 0-7 += Whi^T@(xhi+xlo); rows 32-39 += Wlo^T@(..)
                    h40 = ps_h.tile([WP, 512], f32)
                    first = True
                    for dt in range(D_TILES):
                        for hl in range(2):
                            base = (dt * 2 + hl) * GTOK + half * 512
                            nc.tensor.matmul(
                                h40[:],
                                lhsT=wpair_sb[:, dt * WP : (dt + 1) * WP],
                                rhs=xt[:, base : base + 512],
                                start=first,
                                stop=False,
                            )
                            first = False
                    nc.tensor.matmul(
                        h40[0:K, :],
                        lhsT=b_sb[:],
                        rhs=ones_row[:],
                        start=False,
                        stop=True,
                    )

                    # h = rows[0:8] + rows[32:40]; val = [h >= T] + [h > -T]
                    hlo_sb = val_pool.tile([K, 512], f32, name="hlo_sb")
                    nc.vector.tensor_copy(hlo_sb[:], h40[32 : 32 + K, :])
                    hsum = val_pool.tile([K, 512], f32, name="hsum")
                    nc.vector.tensor_add(hsum[:], h40[0:K, :], hlo_sb[:])
                    val1 = val_pool.tile([K, 512], f32, name="val1")
                    nc.vector.tensor_scalar(
                        out=val1[:],
                        in0=hsum[:],
                        scalar1=T_HI,
                        scalar2=None,
                        op0=mybir.AluOpType.is_ge,
                    )
                    val = val_pool.tile([K, 512], f32, name="val")
                    nc.vector.scalar_tensor_tensor(
                        out=val[:],
                        in0=hsum[:],
                        scalar=-T_HI,
                        in1=val1[:],
                        op0=mybir.AluOpType.is_gt,
                        op1=mybir.AluOpType.add,
                    )
                    # mu = powers^T @ val   (K=8 contraction)
                    mu_ps = ps_m.tile([1, 512], f32)
                    nc.tensor.matmul(
                        mu_ps[:], lhsT=pw_sb[:], rhs=val[:], start=True, stop=True
                    )
                    tbase = g * GTOK + half * 512
                    nc.vector.tensor_copy(
                        mu_i32[:, tbase : tbase + 512], mu_ps[:]
                    )

        nc.sync.dma_start(out_ap[:], mu_i32[:])

    nc.compile()
    return nc


def _build_fp16x2t2(repeat=1):
    """v3: host-pretransposed fp16x2 with software-pipelined mu matmuls.

    Host layout [p, g, half, dt, hl, t512]: each 512-token half is one
    contiguous 2.6MB DMA (20KB/partition row).  Per group, both halves'
    GEMMs accumulate into one [40,1024] PSUM tile (2 banks); the vector
    epilogue runs 1024-wide; the tiny mu matmul for group g is emitted
    after group g+1's main matmuls so the tensor queue never waits on
    the vector engine.  PSUM->SBUF copies ride the idle scalar engine."""
    from contextlib import ExitStack

    from concourse import bacc, mybir, tile

    f16 = mybir.dt.float16
    f32 = mybir.dt.float32
    i32 = mybir.dt.int32

    nc = bacc.Bacc("TRN2", target_bir_lowering=False, debug=False)

    HFREE = D_TILES * 2 * 512           # 10240 per half
    xt_ap = nc.dram_tensor(
        "xt", [128, N_GROUP * 2 * HFREE], f16, kind="ExternalInput"
    ).ap()
    WP = 40
    wpair_ap = nc.dram_tensor(
        "wpair", [128, D_TILES * WP], f16, kind="ExternalInput"
    ).ap()
    b2_ap = nc.dram_tensor("b2", [K, 1], f32, kind="ExternalInput").ap()
    # 3^k split into two fp16-exact columns (3^7=2187 = 2048+139)
    pwa_ap = nc.dram_tensor("pwa", [K, 1], f16, kind="ExternalInput").ap()
    pwb_ap = nc.dram_tensor("pwb", [K, 1], f16, kind="ExternalInput").ap()
    out_ap = nc.dram_tensor(
        "out", [1, TOK_PER_CORE], i32, kind="ExternalOutput"
    ).ap()

    T_HI = T_POS * SPLIT_SCALE * SPLIT_SCALE

    with tile.TileContext(nc) as tc, ExitStack() as ctx:
        const_pool = ctx.enter_context(tc.tile_pool(name="const", bufs=1))
        xt_pool = ctx.enter_context(tc.tile_pool(name="xt", bufs=3))
        val_pool = ctx.enter_context(tc.tile_pool(name="val", bufs=3))
        mu_pool = ctx.enter_context(tc.tile_pool(name="mu", bufs=1))
        ps_h = ctx.enter_context(tc.tile_pool(name="ps_h", bufs=2, space="PSUM"))
        ps_m = ctx.enter_context(tc.tile_pool(name="ps_m", bufs=2, space="PSUM"))

        from concourse.tile_rust import add_dep_helper

        # consts dispatch first (tiny, land in ~1us); group 0's first half
        # is split into per-dt-pair sub-DMAs so matmuls start on dt0 while
        # the rest streams in
        wpair_sb = const_pool.tile([128, D_TILES * WP], f16)
        nc.sync.dma_start(wpair_sb[:], wpair_ap[:])
        b2_sb = const_pool.tile([K, 1], f32)
        nc.sync.dma_start(b2_sb[:], b2_ap[:])
        pwa_sb = const_pool.tile([K, 1], f16)
        nc.sync.dma_start(pwa_sb[:], pwa_ap[:])
        pwb_sb = const_pool.tile([K, 1], f16)
        nc.sync.dma_start(pwb_sb[:], pwb_ap[:])

        xh_tiles = {}

        def issue_xh(g, half, pieces=1):
            t = xt_pool.tile([128, HFREE], f16, name=f"xh{half}")
            off = (g * 2 + half) * HFREE
            step = HFREE // pieces
            for i in range(pieces):
                nc.sync.dma_start(
                    t[:, i * step : (i + 1) * step],
                    xt_ap[:, off + i * step : off + (i + 1) * step],
                )
            xh_tiles[(g, half)] = t

        issue_xh(0, 0, pieces=5)
        issue_xh(0, 1, pieces=2)

        mu_i32 = mu_pool.tile([1, TOK_PER_CORE], i32)

        def emit_mu(val_t, g, half, anchor=None):
            mu_ps = ps_m.tile([1, 512], f32, name=f"mu{half}")
            mm1 = nc.tensor.matmul(
                mu_ps[:], lhsT=pwa_sb[:], rhs=val_t[:], start=True, stop=False
            )
            if anchor is not None:
                # ordering-only edge: keep the scheduler from hoisting the
                # mu matmul ahead of the next half's stream (it would stall
                # the in-order tensor queue waiting on the vector epilogue)
                add_dep_helper(
                    mm1.ins, anchor.ins, sync=False, reason="delay mu"
                )
            nc.tensor.matmul(
                mu_ps[:], lhsT=pwb_sb[:], rhs=val_t[:], start=False, stop=True
            )
            tbase = g * GTOK + half * 512
            nc.scalar.copy(mu_i32[:, tbase : tbase + 512], mu_ps[:])

        for _rep in range(repeat):
            pending = []
            for g in range(N_GROUP):
                if g + 1 < N_GROUP:
                    issue_xh(g + 1, 0)
                    issue_xh(g + 1, 1)
                for half in range(2):
                    xh = xh_tiles.pop((g, half))
                    h40 = ps_h.tile([WP, 512], f32, name=f"h{half}")
                    first = True
                    anchor = None
                    for dt in range(D_TILES):
                        for hl in range(2):
                            base = (dt * 2 + hl) * 512
                            mm = nc.tensor.matmul(
                                h40[:],
                                lhsT=wpair_sb[:, dt * WP : (dt + 1) * WP],
                                rhs=xh[:, base : base + 512],
                                start=first,
                                stop=(dt == D_TILES - 1 and hl == 1),
                            )
                            first = False
                            if dt == 7 and hl == 0:
                                anchor = mm
                    # tensor queue: flush mu of the previous half (its
                    # epilogue ran during this half's matmuls)
                    while pending:
                        emit_mu(*pending.pop(0), anchor=anchor)

                    # epilogue: h = rows[0:8]+(rows[32:40]+b);
                    # val = [h >= T] + [h > -T]   (fp16, for fast mu matmul)
                    hlo_b = val_pool.tile([K, 512], f32, name=f"hlo_b{half}")
                    nc.vector.tensor_scalar(
                        out=hlo_b[:],
                        in0=h40[32 : 32 + K, :],
                        scalar1=b2_sb[:],
                        scalar2=None,
                        op0=mybir.AluOpType.add,
                    )
                    hsum = val_pool.tile([K, 512], f32, name=f"hsum{half}")
                    nc.vector.tensor_add(hsum[:], h40[0:K, :], hlo_b[:])
                    val1 = val_pool.tile([K, 512], f32, name=f"val1{half}")
                    nc.vector.tensor_scalar(
                        out=val1[:],
                        in0=hsum[:],
                        scalar1=T_HI,
                        scalar2=None,
                        op0=mybir.AluOpType.is_ge,
                    )
                    val = val_pool.tile([K, 512], f16, name=f"val{half}")
                    nc.vector.scalar_tensor_tensor(
                        out=val[:],
                        in0=hsum[:],
                        scalar=-T_HI,
                        in1=val1[:],
                        op0=mybir.AluOpType.is_gt,
                        op1=mybir.AluOpType.add,
                    )
                    pending.append((val, g, half))
            while pending:
                emit_mu(*pending.pop(0))

        nc.sync.dma_start(out_ap[:], mu_i32[:])

    nc.compile()
    return nc


def _build_fp32(repeat=1, stages="full"):
    from contextlib import ExitStack

    from concourse import bacc, masks, mybir, tile

    do_t = stages in ("full", "dma_t")
    do_mm = stages in ("full", "dma_mm")

    f32 = mybir.dt.float32
    i32 = mybir.dt.int32

    nc = bacc.Bacc("TRN2", target_bir_lowering=False, debug=False)

    x_ap = nc.dram_tensor("x", [TOK_PER_CORE, D], f32, kind="ExternalInput").ap()
    wt_ap = nc.dram_tensor("wt", [D, K], f32, kind="ExternalInput").ap()
    b_ap = nc.dram_tensor("bias", [1, K], f32, kind="ExternalInput").ap()
    pw_ap = nc.dram_tensor("powers", [K, 1], f32, kind="ExternalInput").ap()
    out_ap = nc.dram_tensor(
        "out", [1, TOK_PER_CORE], i32, kind="ExternalOutput"
    ).ap()
    probe_ap = None
    if stages != "full":
        probe_ap = nc.dram_tensor(
            "probe_out", [128, 8], f32, kind="ExternalOutput"
        ).ap()

    with tile.TileContext(nc) as tc, ExitStack() as ctx:
        const_pool = ctx.enter_context(tc.tile_pool(name="const", bufs=1))
        x_pool = ctx.enter_context(tc.tile_pool(name="x", bufs=4))
        xt_pool = ctx.enter_context(tc.tile_pool(name="xt", bufs=2))
        val_pool = ctx.enter_context(tc.tile_pool(name="val", bufs=3))
        mu_pool = ctx.enter_context(tc.tile_pool(name="mu", bufs=1))
        ps_t = ctx.enter_context(tc.tile_pool(name="ps_t", bufs=5, space="PSUM"))
        ps_h = ctx.enter_context(tc.tile_pool(name="ps_h", bufs=2, space="PSUM"))
        ps_m = ctx.enter_context(tc.tile_pool(name="ps_m", bufs=1, space="PSUM"))

        identity = const_pool.tile([128, 128], f32)
        masks.make_identity(nc, identity[:])

        wt_sb = const_pool.tile([128, D_TILES * K], f32)
        nc.sync.dma_start(
            wt_sb[:].rearrange("p (dt k) -> p dt k", dt=D_TILES),
            wt_ap.rearrange("(dt p) k -> p dt k", p=128),
        )
        b_sb = const_pool.tile([1, K], f32)
        nc.sync.dma_start(b_sb[:], b_ap[:])
        pw_sb = const_pool.tile([K, 1], f32)
        nc.sync.dma_start(pw_sb[:], pw_ap[:])
        ones_row = const_pool.tile([1, TOKS], f32)
        nc.vector.memset(ones_row[:], 1.0)

        mu_i32 = mu_pool.tile([1, TOK_PER_CORE], i32)
        nc.vector.memset(mu_i32[:], 0)

        probe = None
        if stages != "full":
            probe = mu_pool.tile([128, 8], f32, name="probe")

        xt_const = None
        if stages == "dma_mm":
            xt_const = const_pool.tile([128, D_TILES * TOKS], f32)
            nc.vector.memset(xt_const[:], 0.25)

        for _rep in range(repeat):
            for st in range(N_SUPER):
                xs = x_pool.tile([128, SUPER * D], f32)
                nc.sync.dma_start(
                    xs[:].rearrange("p (j d) -> p j d", j=SUPER),
                    x_ap.rearrange("(s j p) d -> s p j d", j=SUPER, p=128)[st],
                )
                xt = None
                if do_t:
                    xt = xt_pool.tile([128, D_TILES * TOKS], f32)
                    ci = 0
                    for j in range(SUPER):
                        for dt in range(D_TILES):
                            ps = ps_t.tile([128, 128], f32)
                            nc.tensor.transpose(
                                ps[:],
                                xs[:, j * D + dt * 128 : j * D + (dt + 1) * 128],
                                identity[:],
                            )
                            dst = xt[
                                :, dt * TOKS + j * 128 : dt * TOKS + (j + 1) * 128
                            ]
                            if ci % 9 < 4:
                                nc.scalar.copy(dst, ps[:])
                            else:
                                nc.vector.tensor_copy(dst, ps[:])
                            ci += 1
                if stages == "dma_mm":
                    xt = xt_const
                    nc.vector.tensor_copy(probe[:], xs[:, 0:8])
                if not do_mm:
                    src = xt if do_t else xs
                    nc.vector.tensor_copy(probe[:], src[:, 0:8])
                    continue

                h = ps_h.tile([K, TOKS], f32)
                for dt in range(D_TILES):
                    nc.tensor.matmul(
                        h[:],
                        lhsT=wt_sb[:, dt * K : (dt + 1) * K],
                        rhs=xt[:, dt * TOKS : (dt + 1) * TOKS],
                        start=(dt == 0),
                        stop=False,
                    )
                nc.tensor.matmul(
                    h[:], lhsT=b_sb[:], rhs=ones_row[:], start=False, stop=True
                )

                val1 = val_pool.tile([K, TOKS], f32)
                nc.vector.tensor_scalar(
                    out=val1[:],
                    in0=h[:],
                    scalar1=T_POS,
                    scalar2=None,
                    op0=mybir.AluOpType.is_ge,
                )
                val = val_pool.tile([K, TOKS], f32)
                nc.vector.scalar_tensor_tensor(
                    out=val[:],
                    in0=h[:],
                    scalar=-T_POS,
                    in1=val1[:],
                    op0=mybir.AluOpType.is_gt,
                    op1=mybir.AluOpType.add,
                )
                mu_ps = ps_m.tile([1, TOKS], f32)
                nc.tensor.matmul(
                    mu_ps[:], lhsT=pw_sb[:], rhs=val[:], start=True, stop=True
                )
                nc.vector.tensor_copy(
                    mu_i32[:, st * TOKS : (st + 1) * TOKS], mu_ps[:]
                )

        nc.sync.dma_start(out_ap[:], mu_i32[:])
        if probe_ap is not None:
            nc.sync.dma_start(probe_ap[:], probe[:])

    nc.compile()
    return nc


def _build_program(repeat=1, stages="full", mode=None):
    mode = mode or MODE
    if mode == "fp16x2t2":
        assert stages == "full"
        return _build_fp16x2t2(repeat)
    if mode == "fp16x2t":
        assert stages == "full"
        return _build_fp16x2t(repeat)
    if mode == "fp16x2":
        assert stages == "full"
        return _build_fp16x2(repeat)
    return _build_fp32(repeat, stages)


def _get_program(repeat=1, mode=None):
    key = ("nc", repeat, mode or MODE)
    if key not in _cached:
        _cached[key] = _build_program(repeat, mode=mode)
    return _cached[key]


def _split_f16(a32):
    hi = a32.astype(np.float16)
    lo = (a32 - hi.astype(np.float32)).astype(np.float16)
    return hi, lo


def make_in_maps(x, W, b, mode=None):
    mode = mode or MODE
    xf = np.ascontiguousarray(x.reshape(-1, D), dtype=np.float32)
    b1 = np.ascontiguousarray(b.reshape(1, K), dtype=np.float32)
    powers = (3.0 ** np.arange(K, dtype=np.float32)).reshape(K, 1).astype(np.float32)
    if mode in ("fp16x2t", "fp16x2t2"):
        xs = xf * np.float32(SPLIT_SCALE)
        xhi, xlo = _split_f16(xs)
        ws = np.ascontiguousarray(W.T, dtype=np.float32) * np.float32(SPLIT_SCALE)
        wthi, wtlo = _split_f16(ws)
        bs = b1 * np.float32(SPLIT_SCALE * SPLIT_SCALE)
        if mode == "fp16x2t":
            # [core*g*t, dt*p] -> [core][p, g, dt, hl, t] contiguous
            r = lambda a: a.reshape(N_CORES, N_GROUP, GTOK, D_TILES, 128)
            pair = np.stack([r(xhi), r(xlo)], axis=3)  # [c, g, t, hl, dt, p]
            pair = np.ascontiguousarray(pair.transpose(0, 5, 1, 4, 3, 2))
            pair = pair.reshape(N_CORES, 128, N_GROUP * D_TILES * 2 * GTOK)
            return [
                {
                    "xt": pair[c],
                    "wthi": wthi,
                    "wtlo": wtlo,
                    "bias": bs,
                    "powers": powers,
                }
                for c in range(N_CORES)
            ]
        # fp16x2t2: [core][p, g, half, dt, hl, t512] contiguous
        r = lambda a: a.reshape(N_CORES, N_GROUP, 2, 512, D_TILES, 128)
        pair = np.stack([r(xhi), r(xlo)], axis=4)  # [c,g,half,t,hl,dt,p]
        pair = np.ascontiguousarray(pair.transpose(0, 6, 1, 2, 5, 4, 3))
        pair = pair.reshape(N_CORES, 128, N_GROUP * D_TILES * 2 * GTOK)
        # prepacked stationary: per d-tile 40 cols, [0:8]=Whi, [32:40]=Wlo
        WP = 40
        wpair = np.zeros((128, D_TILES * WP), dtype=np.float16)
        wr = lambda a: a.reshape(D_TILES, 128, K).transpose(1, 0, 2)
        wpair.reshape(128, D_TILES, WP)[:, :, 0:K] = wr(wthi)
        wpair.reshape(128, D_TILES, WP)[:, :, 32 : 32 + K] = wr(wtlo)
        b2 = np.ascontiguousarray(bs.reshape(K, 1))
        pwa = np.array([1, 3, 9, 27, 81, 243, 729, 2048], dtype=np.float16)
        pwb = np.array([0, 0, 0, 0, 0, 0, 0, 139], dtype=np.float16)
        return [
            {
                "xt": pair[c],
                "wpair": wpair,
                "b2": b2,
                "pwa": pwa.reshape(K, 1),
                "pwb": pwb.reshape(K, 1),
            }
            for c in range(N_CORES)
        ]
    if mode == "fp16x2":
        xs = xf * np.float32(SPLIT_SCALE)
        xhi, xlo = _split_f16(xs)
        ws = np.ascontiguousarray(W.T, dtype=np.float32) * np.float32(SPLIT_SCALE)
        wthi, wtlo = _split_f16(ws)
        bs = b1 * np.float32(SPLIT_SCALE * SPLIT_SCALE)
        return [
            {
                "xhi": xhi[c * TOK_PER_CORE : (c + 1) * TOK_PER_CORE],
                "xlo": xlo[c * TOK_PER_CORE : (c + 1) * TOK_PER_CORE],
                "wthi": wthi,
                "wtlo": wtlo,
                "bias": bs,
                "powers": powers,
            }
            for c in range(N_CORES)
        ]
    wt = np.ascontiguousarray(W.T, dtype=np.float32)
    return [
        {
            "x": xf[c * TOK_PER_CORE : (c + 1) * TOK_PER_CORE],
            "wt": wt,
            "bias": b1,
            "powers": powers,
        }
        for c in range(N_CORES)
    ]


def kernel(x: np.ndarray, W: np.ndarray, b: np.ndarray) -> np.ndarray:
    from concourse.bass_utils import run_bass_kernel_spmd

    nc = _get_program()

    B, T, Dx = x.shape
    assert (B * T, Dx) == (N_CORES * TOK_PER_CORE, D)
    in_maps = make_in_maps(x, W, b)
    res = run_bass_kernel_spmd(nc, in_maps, list(range(N_CORES)))
    mu = np.concatenate(
        [res.results[c]["out"].reshape(-1) for c in range(N_CORES)]
    )
    return mu.reshape(B, T).astype(np.int32)



# revision 24
# speedup vs baseline: 1.1190x; 1.1190x over previous
"""FSQ codebook kernel for Trainium2 (8 NeuronCores, data-parallel over tokens).

Computes, for x:(8,8192,1280) f32, W:(8,1280) f32, b:(8,) f32:
    h  = x.reshape(-1,1280) @ W.T + b            # (65536, 8)
    mu = sum_k 3^k * (1 + round(tanh(h)*SCALE))  # base-3 code, int32
    -> (8, 8192) int32

The tanh/round/scale pipeline is replaced by an exact fp32 threshold:
    round(tanh(h)*SCALE) = +1  iff  h >= T_POS
                         = -1  iff  h <= -T_POS      (bit-exact, verified)
so digit value (1+r) = [h >= T_POS] + [h > -T_POS] and
    mu = sum_k 3^k*[h_k >= T] + sum_k 3^k*[h_k > -T].

fp16x2 path (default): x and W are scaled by 2^10 and Dekker-split on the
host into hi/lo fp16 pairs (x*2^10 = hi + lo + O(2^-24); same total bytes).
The 16-bit XBAR DMA-transpose loads x already transposed (d on partitions),
eliminating all PE transposes and PSUM->SBUF copies. The GEMM is 4 fp16
matmul products per d-tile at full PE rate, accumulating the 2^20-scaled h
in fp32 PSUM; thresholds are scaled by exactly 2^20 (power-of-two => the
comparison is unchanged).

fp32 path (fallback): PE-transpose 128x128 blocks + ACT/DVE copies + fp32
matmuls (weight stationary).
"""

import numpy as np

# exact fp32 threshold: minimal fp32 v with round(tanh(v)*SCALE) == 1
T_POS = float(np.uint32(0x3F0CCB15).view(np.float32))
SPLIT_SCALE = 1024.0  # 2^10 per operand; h is scaled by 2^20

N_CORES = 8
TOK_PER_CORE = 8192
D = 1280
K = 8
D_TILES = D // 128            # 10

MODE = "fp16x2t2"
N_PRODUCTS = 4                # hi*hi, hi*lo, lo*hi, lo*lo

# fp32-path tiling
SUPER = 4
TOKS = SUPER * 128            # 512
N_SUPER = TOK_PER_CORE // TOKS

# fp16x2-path tiling: 1024-token groups, matmul N=512 halves
GTOK = 1024
N_GROUP = TOK_PER_CORE // GTOK  # 8

_cached = {}


def _build_fp16x2(repeat=1):
    from contextlib import ExitStack

    from concourse import bacc, mybir, tile

    f16 = mybir.dt.float16
    f32 = mybir.dt.float32
    i32 = mybir.dt.int32

    nc = bacc.Bacc("TRN2", target_bir_lowering=False, debug=False)

    xhi_ap = nc.dram_tensor("xhi", [TOK_PER_CORE, D], f16, kind="ExternalInput").ap()
    xlo_ap = nc.dram_tensor("xlo", [TOK_PER_CORE, D], f16, kind="ExternalInput").ap()
    wthi_ap = nc.dram_tensor("wthi", [D, K], f16, kind="ExternalInput").ap()
    wtlo_ap = nc.dram_tensor("wtlo", [D, K], f16, kind="ExternalInput").ap()
    b_ap = nc.dram_tensor("bias", [1, K], f32, kind="ExternalInput").ap()
    pw_ap = nc.dram_tensor("powers", [K, 1], f32, kind="ExternalInput").ap()
    out_ap = nc.dram_tensor(
        "out", [1, TOK_PER_CORE], i32, kind="ExternalOutput"
    ).ap()

    T_HI = T_POS * SPLIT_SCALE * SPLIT_SCALE

    with tile.TileContext(nc) as tc, ExitStack() as ctx:
        const_pool = ctx.enter_context(tc.tile_pool(name="const", bufs=1))
        xt_pool = ctx.enter_context(tc.tile_pool(name="xt", bufs=3))
        val_pool = ctx.enter_context(tc.tile_pool(name="val", bufs=3))
        mu_pool = ctx.enter_context(tc.tile_pool(name="mu", bufs=1))
        ps_h = ctx.enter_context(tc.tile_pool(name="ps_h", bufs=4, space="PSUM"))
        ps_m = ctx.enter_context(tc.tile_pool(name="ps_m", bufs=2, space="PSUM"))

        # stacked stationary, 40 cols per d-tile: cols [0:8]=Whi_dt,
        # [32:40]=Wlo_dt (partition windows must start at multiples of 32;
        # the unused middle columns cost nothing — matmul time is N-bound)
        WP = 40
        wpair_sb = const_pool.tile([128, D_TILES * WP], f16)
        nc.vector.memset(wpair_sb[:], 0)
        nc.sync.dma_start(
            wpair_sb[:].rearrange("p (dt c) -> p dt c", dt=D_TILES)[:, :, 0:K],
            wthi_ap.rearrange("(dt p) k -> p dt k", p=128),
        )
        nc.sync.dma_start(
            wpair_sb[:].rearrange("p (dt c) -> p dt c", dt=D_TILES)[
                :, :, 32 : 32 + K
            ],
            wtlo_ap.rearrange("(dt p) k -> p dt k", p=128),
        )
        b_sb = const_pool.tile([1, K], f32)
        nc.sync.dma_start(b_sb[:], b_ap[:])
        pw_sb = const_pool.tile([K, 1], f32)
        nc.sync.dma_start(pw_sb[:], pw_ap[:])
        ones_row = const_pool.tile([1, 512], f32)
        nc.vector.memset(ones_row[:], 1.0)

        mu_i32 = mu_pool.tile([1, TOK_PER_CORE], i32)

        for _rep in range(repeat):
            for g in range(N_GROUP):
                t0 = g * GTOK
                # DMA-transpose loads: xthi[d, dt*GTOK + t] = xhi[t0+t, dt*128+d]
                xthi = xt_pool.tile([128, D_TILES * GTOK], f16, name="xthi")
                xtlo = xt_pool.tile([128, D_TILES * GTOK], f16, name="xtlo")
                for dt in range(D_TILES):
                    nc.sync.dma_start(
                        xthi[:, dt * GTOK : (dt + 1) * GTOK],
                        xhi_ap[t0 : t0 + GTOK, dt * 128 : (dt + 1) * 128],
                        transpose=True,
                    )
                    nc.sync.dma_start(
                        xtlo[:, dt * GTOK : (dt + 1) * GTOK],
                        xlo_ap[t0 : t0 + GTOK, dt * 128 : (dt + 1) * 128],
                        transpose=True,
                    )
                for half in range(2):
                    sl = lambda dt: slice(
                        dt * GTOK + half * 512, dt * GTOK + half * 512 + 512
                    )
                    # h40 rows 0-7 += Whi^T@(xthi+xtlo); rows 32-39 += Wlo^T@(...)
                    # all 4 Dekker products in 2 matmuls per d-tile
                    h40 = ps_h.tile([WP, 512], f32)
                    first = True
                    for dt in range(D_TILES):
                        for xsb in (xthi, xtlo):
                            nc.tensor.matmul(
                                h40[:],
                                lhsT=wpair_sb[:, dt * WP : (dt + 1) * WP],
                                rhs=xsb[:, sl(dt)],
                                start=first,
                                stop=False,
                            )
                            first = False
                    nc.tensor.matmul(
                        h40[0:K, :],
                        lhsT=b_sb[:],
                        rhs=ones_row[:],
                        start=False,
                        stop=True,
                    )

                    # h = rows[0:8] + rows[32:40]; val = [h >= T] + [h > -T]
                    # (tensor_tensor may read only one PSUM operand)
                    hlo_sb = val_pool.tile([K, 512], f32, name="hlo_sb")
                    nc.vector.tensor_copy(hlo_sb[:], h40[32 : 32 + K, :])
                    hsum = val_pool.tile([K, 512], f32, name="hsum")
                    nc.vector.tensor_add(hsum[:], h40[0:K, :], hlo_sb[:])
                    val1 = val_pool.tile([K, 512], f32, name="val1")
                    nc.vector.tensor_scalar(
                        out=val1[:],
                        in0=hsum[:],
                        scalar1=T_HI,
                        scalar2=None,
                        op0=mybir.AluOpType.is_ge,
                    )
                    val = val_pool.tile([K, 512], f32, name="val")
                    nc.vector.scalar_tensor_tensor(
                        out=val[:],
                        in0=hsum[:],
                        scalar=-T_HI,
                        in1=val1[:],
                        op0=mybir.AluOpType.is_gt,
                        op1=mybir.AluOpType.add,
                    )
                    # mu = powers^T @ val   (K=8 contraction)
                    mu_ps = ps_m.tile([1, 512], f32)
                    nc.tensor.matmul(
                        mu_ps[:], lhsT=pw_sb[:], rhs=val[:], start=True, stop=True
                    )
                    base = t0 + half * 512
                    nc.vector.tensor_copy(
                        mu_i32[:, base : base + 512], mu_ps[:]
                    )

        nc.sync.dma_start(out_ap[:], mu_i32[:])

    nc.compile()
    return nc


def _build_fp16x2t(repeat=1):
    """Host-pretransposed fp16x2: x is uploaded as one [128, G*DT*2*GTOK]
    fp16 tensor laid out [p, g, dt, hl, t] so each 1024-token group is a
    single fully-contiguous 5.2MB DMA (40KB/partition row).  No XBAR
    transpose, no per-tile DMA — the load runs at full HBM bandwidth and
    fans out across all 16 HW DGE engines automatically."""
    from contextlib import ExitStack

    from concourse import bacc, mybir, tile

    f16 = mybir.dt.float16
    f32 = mybir.dt.float32
    i32 = mybir.dt.int32

    nc = bacc.Bacc("TRN2", target_bir_lowering=False, debug=False)

    GFREE = D_TILES * 2 * GTOK          # 20480 per group
    xt_ap = nc.dram_tensor(
        "xt", [128, N_GROUP * GFREE], f16, kind="ExternalInput"
    ).ap()
    wthi_ap = nc.dram_tensor("wthi", [D, K], f16, kind="ExternalInput").ap()
    wtlo_ap = nc.dram_tensor("wtlo", [D, K], f16, kind="ExternalInput").ap()
    b_ap = nc.dram_tensor("bias", [1, K], f32, kind="ExternalInput").ap()
    pw_ap = nc.dram_tensor("powers", [K, 1], f32, kind="ExternalInput").ap()
    out_ap = nc.dram_tensor(
        "out", [1, TOK_PER_CORE], i32, kind="ExternalOutput"
    ).ap()

    T_HI = T_POS * SPLIT_SCALE * SPLIT_SCALE

    with tile.TileContext(nc) as tc, ExitStack() as ctx:
        const_pool = ctx.enter_context(tc.tile_pool(name="const", bufs=1))
        xt_pool = ctx.enter_context(tc.tile_pool(name="xt", bufs=3))
        val_pool = ctx.enter_context(tc.tile_pool(name="val", bufs=3))
        mu_pool = ctx.enter_context(tc.tile_pool(name="mu", bufs=1))
        ps_h = ctx.enter_context(tc.tile_pool(name="ps_h", bufs=4, space="PSUM"))
        ps_m = ctx.enter_context(tc.tile_pool(name="ps_m", bufs=2, space="PSUM"))

        # stacked stationary, 40 cols per d-tile: cols [0:8]=Whi_dt,
        # [32:40]=Wlo_dt (partition windows must start at multiples of 32)
        WP = 40
        wpair_sb = const_pool.tile([128, D_TILES * WP], f16)
        nc.vector.memset(wpair_sb[:], 0)
        nc.sync.dma_start(
            wpair_sb[:].rearrange("p (dt c) -> p dt c", dt=D_TILES)[:, :, 0:K],
            wthi_ap.rearrange("(dt p) k -> p dt k", p=128),
        )
        nc.sync.dma_start(
            wpair_sb[:].rearrange("p (dt c) -> p dt c", dt=D_TILES)[
                :, :, 32 : 32 + K
            ],
            wtlo_ap.rearrange("(dt p) k -> p dt k", p=128),
        )
        b_sb = const_pool.tile([1, K], f32)
        nc.sync.dma_start(b_sb[:], b_ap[:])
        pw_sb = const_pool.tile([K, 1], f32)
        nc.sync.dma_start(pw_sb[:], pw_ap[:])
        ones_row = const_pool.tile([1, 512], f32)
        nc.vector.memset(ones_row[:], 1.0)

        mu_i32 = mu_pool.tile([1, TOK_PER_CORE], i32)

        for _rep in range(repeat):
            for g in range(N_GROUP):
                xt = xt_pool.tile([128, GFREE], f16, name="xt")
                nc.sync.dma_start(
                    xt[:], xt_ap[:, g * GFREE : (g + 1) * GFREE]
                )
                for half in range(2):
                    # h40 rows 0-7 += Whi^T@(xhi+xlo); rows 32-39 += Wlo^T@(..)
                    h40 = ps_h.tile([WP, 512], f32)
                    first = True
                    for dt in range(D_TILES):
                        for hl in range(2):
                            base = (dt * 2 + hl) * GTOK + half * 512
                            nc.tensor.matmul(
                                h40[:],
                                lhsT=wpair_sb[:, dt * WP : (dt + 1) * WP],
                                rhs=xt[:, base : base + 512],
                                start=first,
                                stop=False,
                            )
                            first = False
                    nc.tensor.matmul(
                        h40[0:K, :],
                        lhsT=b_sb[:],
                        rhs=ones_row[:],
                        start=False,
                        stop=True,
                    )

                    # h = rows[0:8] + rows[32:40]; val = [h >= T] + [h > -T]
                    hlo_sb = val_pool.tile([K, 512], f32, name="hlo_sb")
                    nc.vector.tensor_copy(hlo_sb[:], h40[32 : 32 + K, :])
                    hsum = val_pool.tile([K, 512], f32, name="hsum")
                    nc.vector.tensor_add(hsum[:], h40[0:K, :], hlo_sb[:])
                    val1 = val_pool.tile([K, 512], f32, name="val1")
                    nc.vector.tensor_scalar(
                        out=val1[:],
                        in0=hsum[:],
                        scalar1=T_HI,
                        scalar2=None,
                        op0=mybir.AluOpType.is_ge,
                    )
                    val = val_pool.tile([K, 512], f32, name="val")
                    nc.vector.scalar_tensor_tensor(
                        out=val[:],
                        in0=hsum[:],
                        scalar=-T_HI,
                        in1=val1[:],
                        op0=mybir.AluOpType.is_gt,
                        op1=mybir.AluOpType.add,
                    )
                    # mu = powers^T @ val   (K=8 contraction)
                    mu_ps = ps_m.tile([1, 512], f32)
                    nc.tensor.matmul(
                        mu_ps[:], lhsT=pw_sb[:], rhs=val[:], start=True, stop=True
                    )
                    tbase = g * GTOK + half * 512
                    nc.vector.tensor_copy(
                        mu_i32[:, tbase : tbase + 512], mu_ps[:]
                    )

        nc.sync.dma_start(out_ap[:], mu_i32[:])

    nc.compile()
    return nc


def _build_fp16x2t2(repeat=1):
    """v3: host-pretransposed fp16x2 with software-pipelined mu matmuls.

    Host layout [p, g, half, dt, hl, t512]: each 512-token half is one
    contiguous 2.6MB DMA (20KB/partition row).  Per group, both halves'
    GEMMs accumulate into one [40,1024] PSUM tile (2 banks); the vector
    epilogue runs 1024-wide; the tiny mu matmul for group g is emitted
    after group g+1's main matmuls so the tensor queue never waits on
    the vector engine.  PSUM->SBUF copies ride the idle scalar engine."""
    from contextlib import ExitStack

    from concourse import bacc, mybir, tile

    f16 = mybir.dt.float16
    f32 = mybir.dt.float32
    i32 = mybir.dt.int32

    nc = bacc.Bacc("TRN2", target_bir_lowering=False, debug=False)

    HFREE = D_TILES * 2 * 512           # 10240 per half
    xt_ap = nc.dram_tensor(
        "xt", [128, N_GROUP * 2 * HFREE], f16, kind="ExternalInput"
    ).ap()
    WP = 40
    wpair_ap = nc.dram_tensor(
        "wpair", [128, D_TILES * WP], f16, kind="ExternalInput"
    ).ap()
    b2_ap = nc.dram_tensor("b2", [K, 1], f32, kind="ExternalInput").ap()
    # 3^k split into two fp16-exact columns (3^7=2187 = 2048+139)
    pwa_ap = nc.dram_tensor("pwa", [K, 1], f16, kind="ExternalInput").ap()
    pwb_ap = nc.dram_tensor("pwb", [K, 1], f16, kind="ExternalInput").ap()
    out_ap = nc.dram_tensor(
        "out", [1, TOK_PER_CORE], i32, kind="ExternalOutput"
    ).ap()

    T_HI = T_POS * SPLIT_SCALE * SPLIT_SCALE

    with tile.TileContext(nc) as tc, ExitStack() as ctx:
        const_pool = ctx.enter_context(tc.tile_pool(name="const", bufs=1))
        xt_pool = ctx.enter_context(tc.tile_pool(name="xt", bufs=4))
        val_pool = ctx.enter_context(tc.tile_pool(name="val", bufs=2))
        mu_pool = ctx.enter_context(tc.tile_pool(name="mu", bufs=2))
        ps_h = ctx.enter_context(tc.tile_pool(name="ps_h", bufs=2, space="PSUM"))
        ps_m = ctx.enter_context(tc.tile_pool(name="ps_m", bufs=2, space="PSUM"))

        from concourse.tile_rust import add_dep_helper

        # consts ride the scalar HWDGE queue so the sync queue's first
        # dispatch is already x data; group 0's first half is split into
        # per-dt-pair sub-DMAs so matmuls start on dt0 while the rest
        # streams in
        wpair_sb = const_pool.tile([128, D_TILES * WP], f16)
        nc.scalar.dma_start(wpair_sb[:], wpair_ap[:])
        b2_sb = const_pool.tile([K, 1], f32)
        nc.scalar.dma_start(b2_sb[:], b2_ap[:])
        pwa_sb = const_pool.tile([K, 1], f16)
        nc.scalar.dma_start(pwa_sb[:], pwa_ap[:])
        pwb_sb = const_pool.tile([K, 1], f16)
        nc.scalar.dma_start(pwb_sb[:], pwb_ap[:])

        xh_tiles = {}

        def issue_xh(g, half, pieces=1):
            t = xt_pool.tile([128, HFREE], f16, name=f"xh{half}")
            off = (g * 2 + half) * HFREE
            step = HFREE // pieces
            for i in range(pieces):
                nc.sync.dma_start(
                    t[:, i * step : (i + 1) * step],
                    xt_ap[:, off + i * step : off + (i + 1) * step],
                )
            xh_tiles[(g, half)] = t

        issue_xh(0, 0, pieces=5)
        issue_xh(0, 1, pieces=2)

        def emit_mu(val_t, g, half, anchor=None):
            mu_ps = ps_m.tile([1, 512], f32, name=f"mu{half}")
            mm1 = nc.tensor.matmul(
                mu_ps[:], lhsT=pwa_sb[:], rhs=val_t[:], start=True, stop=False
            )
            if anchor is not None:
                # ordering-only edge: keep the scheduler from hoisting the
                # mu matmul ahead of the next half's stream (it would stall
                # the in-order tensor queue waiting on the vector epilogue)
                add_dep_helper(
                    mm1.ins, anchor.ins, sync=False, reason="delay mu"
                )
            nc.tensor.matmul(
                mu_ps[:], lhsT=pwb_sb[:], rhs=val_t[:], start=False, stop=True
            )
            # stage in SBUF (i32 convert) then stream out on the scalar
            # DGE queue; the copy and DMA stay on one queue so no extra
            # cross-engine sems land on the tail
            stage = mu_pool.tile([1, 512], i32, name=f"stage{half}")
            nc.scalar.copy(stage[:], mu_ps[:])
            tbase = g * GTOK + half * 512
            nc.scalar.dma_start(out_ap[:, tbase : tbase + 512], stage[:])

        for _rep in range(repeat):
            pending = []
            for g in range(N_GROUP):
                if g + 1 < N_GROUP:
                    issue_xh(g + 1, 0)
                    issue_xh(g + 1, 1)
                for half in range(2):
                    xh = xh_tiles.pop((g, half))
                    h40 = ps_h.tile([WP, 512], f32, name=f"h{half}")
                    first = True
                    anchor = None
                    for dt in range(D_TILES):
                        for hl in range(2):
                            base = (dt * 2 + hl) * 512
                            mm = nc.tensor.matmul(
                                h40[:],
                                lhsT=wpair_sb[:, dt * WP : (dt + 1) * WP],
                                rhs=xh[:, base : base + 512],
                                start=first,
                                stop=(dt == D_TILES - 1 and hl == 1),
                            )
                            first = False
                            if dt == 7 and hl == 0:
                                anchor = mm
                    # tensor queue: flush mu of the previous half (its
                    # epilogue ran during this half's matmuls)
                    while pending:
                        emit_mu(*pending.pop(0), anchor=anchor)

                    # epilogue: h = rows[0:8]+(rows[32:40]+b);
                    # val = [h >= T] + [h > -T]   (fp16, for fast mu matmul)
                    hlo_b = val_pool.tile([K, 512], f32, name=f"hlo_b{half}")
                    nc.vector.tensor_scalar(
                        out=hlo_b[:],
                        in0=h40[32 : 32 + K, :],
                        scalar1=b2_sb[:],
                        scalar2=None,
                        op0=mybir.AluOpType.add,
                    )
                    hsum = val_pool.tile([K, 512], f32, name=f"hsum{half}")
                    nc.vector.tensor_add(hsum[:], h40[0:K, :], hlo_b[:])
                    val1 = val_pool.tile([K, 512], f32, name=f"val1{half}")
                    nc.vector.tensor_scalar(
                        out=val1[:],
                        in0=hsum[:],
                        scalar1=T_HI,
                        scalar2=None,
                        op0=mybir.AluOpType.is_ge,
                    )
                    val = val_pool.tile([K, 512], f16, name=f"val{half}")
                    nc.vector.scalar_tensor_tensor(
                        out=val[:],
                        in0=hsum[:],
                        scalar=-T_HI,
                        in1=val1[:],
                        op0=mybir.AluOpType.is_gt,
                        op1=mybir.AluOpType.add,
                    )
                    pending.append((val, g, half))
            while pending:
                emit_mu(*pending.pop(0))

    nc.compile()
    return nc


def _build_fp32(repeat=1, stages="full"):
    from contextlib import ExitStack

    from concourse import bacc, masks, mybir, tile

    do_t = stages in ("full", "dma_t")
    do_mm = stages in ("full", "dma_mm")

    f32 = mybir.dt.float32
    i32 = mybir.dt.int32

    nc = bacc.Bacc("TRN2", target_bir_lowering=False, debug=False)

    x_ap = nc.dram_tensor("x", [TOK_PER_CORE, D], f32, kind="ExternalInput").ap()
    wt_ap = nc.dram_tensor("wt", [D, K], f32, kind="ExternalInput").ap()
    b_ap = nc.dram_tensor("bias", [1, K], f32, kind="ExternalInput").ap()
    pw_ap = nc.dram_tensor("powers", [K, 1], f32, kind="ExternalInput").ap()
    out_ap = nc.dram_tensor(
        "out", [1, TOK_PER_CORE], i32, kind="ExternalOutput"
    ).ap()
    probe_ap = None
    if stages != "full":
        probe_ap = nc.dram_tensor(
            "probe_out", [128, 8], f32, kind="ExternalOutput"
        ).ap()

    with tile.TileContext(nc) as tc, ExitStack() as ctx:
        const_pool = ctx.enter_context(tc.tile_pool(name="const", bufs=1))
        x_pool = ctx.enter_context(tc.tile_pool(name="x", bufs=4))
        xt_pool = ctx.enter_context(tc.tile_pool(name="xt", bufs=2))
        val_pool = ctx.enter_context(tc.tile_pool(name="val", bufs=3))
        mu_pool = ctx.enter_context(tc.tile_pool(name="mu", bufs=1))
        ps_t = ctx.enter_context(tc.tile_pool(name="ps_t", bufs=5, space="PSUM"))
        ps_h = ctx.enter_context(tc.tile_pool(name="ps_h", bufs=2, space="PSUM"))
        ps_m = ctx.enter_context(tc.tile_pool(name="ps_m", bufs=1, space="PSUM"))

        identity = const_pool.tile([128, 128], f32)
        masks.make_identity(nc, identity[:])

        wt_sb = const_pool.tile([128, D_TILES * K], f32)
        nc.sync.dma_start(
            wt_sb[:].rearrange("p (dt k) -> p dt k", dt=D_TILES),
            wt_ap.rearrange("(dt p) k -> p dt k", p=128),
        )
        b_sb = const_pool.tile([1, K], f32)
        nc.sync.dma_start(b_sb[:], b_ap[:])
        pw_sb = const_pool.tile([K, 1], f32)
        nc.sync.dma_start(pw_sb[:], pw_ap[:])
        ones_row = const_pool.tile([1, TOKS], f32)
        nc.vector.memset(ones_row[:], 1.0)

        mu_i32 = mu_pool.tile([1, TOK_PER_CORE], i32)
        nc.vector.memset(mu_i32[:], 0)

        probe = None
        if stages != "full":
            probe = mu_pool.tile([128, 8], f32, name="probe")

        xt_const = None
        if stages == "dma_mm":
            xt_const = const_pool.tile([128, D_TILES * TOKS], f32)
            nc.vector.memset(xt_const[:], 0.25)

        for _rep in range(repeat):
            for st in range(N_SUPER):
                xs = x_pool.tile([128, SUPER * D], f32)
                nc.sync.dma_start(
                    xs[:].rearrange("p (j d) -> p j d", j=SUPER),
                    x_ap.rearrange("(s j p) d -> s p j d", j=SUPER, p=128)[st],
                )
                xt = None
                if do_t:
                    xt = xt_pool.tile([128, D_TILES * TOKS], f32)
                    ci = 0
                    for j in range(SUPER):
                        for dt in range(D_TILES):
                            ps = ps_t.tile([128, 128], f32)
                            nc.tensor.transpose(
                                ps[:],
                                xs[:, j * D + dt * 128 : j * D + (dt + 1) * 128],
                                identity[:],
                            )
                            dst = xt[
                                :, dt * TOKS + j * 128 : dt * TOKS + (j + 1) * 128
                            ]
                            if ci % 9 < 4:
                                nc.scalar.copy(dst, ps[:])
                            else:
                                nc.vector.tensor_copy(dst, ps[:])
                            ci += 1
                if stages == "dma_mm":
                    xt = xt_const
                    nc.vector.tensor_copy(probe[:], xs[:, 0:8])
                if not do_mm:
                    src = xt if do_t else xs
                    nc.vector.tensor_copy(probe[:], src[:, 0:8])
                    continue

                h = ps_h.tile([K, TOKS], f32)
                for dt in range(D_TILES):
                    nc.tensor.matmul(
                        h[:],
                        lhsT=wt_sb[:, dt * K : (dt + 1) * K],
                        rhs=xt[:, dt * TOKS : (dt + 1) * TOKS],
                        start=(dt == 0),
                        stop=False,
                    )
                nc.tensor.matmul(
                    h[:], lhsT=b_sb[:], rhs=ones_row[:], start=False, stop=True
                )

                val1 = val_pool.tile([K, TOKS], f32)
                nc.vector.tensor_scalar(
                    out=val1[:],
                    in0=h[:],
                    scalar1=T_POS,
                    scalar2=None,
                    op0=mybir.AluOpType.is_ge,
                )
                val = val_pool.tile([K, TOKS], f32)
                nc.vector.scalar_tensor_tensor(
                    out=val[:],
                    in0=h[:],
                    scalar=-T_POS,
                    in1=val1[:],
                    op0=mybir.AluOpType.is_gt,
                    op1=mybir.AluOpType.add,
                )
                mu_ps = ps_m.tile([1, TOKS], f32)
                nc.tensor.matmul(
                    mu_ps[:], lhsT=pw_sb[:], rhs=val[:], start=True, stop=True
                )
                nc.vector.tensor_copy(
                    mu_i32[:, st * TOKS : (st + 1) * TOKS], mu_ps[:]
                )

        nc.sync.dma_start(out_ap[:], mu_i32[:])
        if probe_ap is not None:
            nc.sync.dma_start(probe_ap[:], probe[:])

    nc.compile()
    return nc


def _build_program(repeat=1, stages="full", mode=None):
    mode = mode or MODE
    if mode == "fp16x2t2":
        assert stages == "full"
        return _build_fp16x2t2(repeat)
    if mode == "fp16x2t":
        assert stages == "full"
        return _build_fp16x2t(repeat)
    if mode == "fp16x2":
        assert stages == "full"
        return _build_fp16x2(repeat)
    return _build_fp32(repeat, stages)


def _get_program(repeat=1, mode=None):
    key = ("nc", repeat, mode or MODE)
    if key not in _cached:
        _cached[key] = _build_program(repeat, mode=mode)
    return _cached[key]


def _split_f16(a32):
    hi = a32.astype(np.float16)
    lo = (a32 - hi.astype(np.float32)).astype(np.float16)
    return hi, lo


def make_in_maps(x, W, b, mode=None):
    mode = mode or MODE
    xf = np.ascontiguousarray(x.reshape(-1, D), dtype=np.float32)
    b1 = np.ascontiguousarray(b.reshape(1, K), dtype=np.float32)
    powers = (3.0 ** np.arange(K, dtype=np.float32)).reshape(K, 1).astype(np.float32)
    if mode in ("fp16x2t", "fp16x2t2"):
        xs = xf * np.float32(SPLIT_SCALE)
        xhi, xlo = _split_f16(xs)
        ws = np.ascontiguousarray(W.T, dtype=np.float32) * np.float32(SPLIT_SCALE)
        wthi, wtlo = _split_f16(ws)
        bs = b1 * np.float32(SPLIT_SCALE * SPLIT_SCALE)
        if mode == "fp16x2t":
            # [core*g*t, dt*p] -> [core][p, g, dt, hl, t] contiguous
            r = lambda a: a.reshape(N_CORES, N_GROUP, GTOK, D_TILES, 128)
            pair = np.stack([r(xhi), r(xlo)], axis=3)  # [c, g, t, hl, dt, p]
            pair = np.ascontiguousarray(pair.transpose(0, 5, 1, 4, 3, 2))
            pair = pair.reshape(N_CORES, 128, N_GROUP * D_TILES * 2 * GTOK)
            return [
                {
                    "xt": pair[c],
                    "wthi": wthi,
                    "wtlo": wtlo,
                    "bias": bs,
                    "powers": powers,
                }
                for c in range(N_CORES)
            ]
        # fp16x2t2: [core][p, g, half, dt, hl, t512] contiguous
        r = lambda a: a.reshape(N_CORES, N_GROUP, 2, 512, D_TILES, 128)
        pair = np.stack([r(xhi), r(xlo)], axis=4)  # [c,g,half,t,hl,dt,p]
        pair = np.ascontiguousarray(pair.transpose(0, 6, 1, 2, 5, 4, 3))
        pair = pair.reshape(N_CORES, 128, N_GROUP * D_TILES * 2 * GTOK)
        # prepacked stationary: per d-tile 40 cols, [0:8]=Whi, [32:40]=Wlo
        WP = 40
        wpair = np.zeros((128, D_TILES * WP), dtype=np.float16)
        wr = lambda a: a.reshape(D_TILES, 128, K).transpose(1, 0, 2)
        wpair.reshape(128, D_TILES, WP)[:, :, 0:K] = wr(wthi)
        wpair.reshape(128, D_TILES, WP)[:, :, 32 : 32 + K] = wr(wtlo)
        b2 = np.ascontiguousarray(bs.reshape(K, 1))
        pwa = np.array([1, 3, 9, 27, 81, 243, 729, 2048], dtype=np.float16)
        pwb = np.array([0, 0, 0, 0, 0, 0, 0, 139], dtype=np.float16)
        return [
            {
                "xt": pair[c],
                "wpair": wpair,
                "b2": b2,
                "pwa": pwa.reshape(K, 1),
                "pwb": pwb.reshape(K, 1),
            }
            for c in range(N_CORES)
        ]
    if mode == "fp16x2":
        xs = xf * np.float32(SPLIT_SCALE)
        xhi, xlo = _split_f16(xs)
        ws = np.ascontiguousarray(W.T, dtype=np.float32) * np.float32(SPLIT_SCALE)
        wthi, wtlo = _split_f16(ws)
        bs = b1 * np.float32(SPLIT_SCALE * SPLIT_SCALE)
        return [
            {
                "xhi": xhi[c * TOK_PER_CORE : (c + 1) * TOK_PER_CORE],
                "xlo": xlo[c * TOK_PER_CORE : (c + 1) * TOK_PER_CORE],
                "wthi": wthi,
                "wtlo": wtlo,
                "bias": bs,
                "powers": powers,
            }
            for c in range(N_CORES)
        ]
    wt = np.ascontiguousarray(W.T, dtype=np.float32)
    return [
        {
            "x": xf[c * TOK_PER_CORE : (c + 1) * TOK_PER_CORE],
            "wt": wt,
            "bias": b1,
            "powers": powers,
        }
        for c in range(N_CORES)
    ]


def kernel(x: np.ndarray, W: np.ndarray, b: np.ndarray) -> np.ndarray:
    from concourse.bass_utils import run_bass_kernel_spmd

    nc = _get_program()

    B, T, Dx = x.shape
    assert (B * T, Dx) == (N_CORES * TOK_PER_CORE, D)
    in_maps = make_in_maps(x, W, b)
    res = run_bass_kernel_spmd(nc, in_maps, list(range(N_CORES)))
    mu = np.concatenate(
        [res.results[c]["out"].reshape(-1) for c in range(N_CORES)]
    )
    return mu.reshape(B, T).astype(np.int32)



# revision 30
# speedup vs baseline: 1.3222x; 1.1816x over previous
"""FSQ codebook kernel for Trainium2 (8 NeuronCores, data-parallel over tokens).

Computes, for x:(8,8192,1280) f32, W:(8,1280) f32, b:(8,) f32:
    h  = x.reshape(-1,1280) @ W.T + b            # (65536, 8)
    mu = sum_k 3^k * (1 + round(tanh(h)*SCALE))  # base-3 code, int32
    -> (8, 8192) int32

The tanh/round/scale pipeline is replaced by an exact fp32 threshold:
    round(tanh(h)*SCALE) = +1  iff  h >= T_POS
                         = -1  iff  h <= -T_POS      (bit-exact, verified)
so digit value (1+r) = [h >= T_POS] + [h > -T_POS] and
    mu = sum_k 3^k*[h_k >= T] + sum_k 3^k*[h_k > -T].

Default path (fp16x2t2): x and W are scaled by 2^10 and Dekker-split on
the host into hi/lo fp16 pairs (x*2^10 = hi + lo + O(2^-24); same total
bytes as the fp32 input, and exact enough that the result is bit-identical
to the fp32 reference).  The host also pre-transposes x into a
partition-major blocked layout [p, group, half, dtile, hi/lo, t512] so
each 512-token half is ONE fully contiguous 1.3MB-per-piece DMA that the
16 HW DGE engines stream at ~410-430 GB/s — this is the whole-kernel
bottleneck (42MB/core of HBM traffic; chip-level HBM roofline).

Per 512-token half, 20 fp16 matmuls (N=512, stacked Whi|Wlo stationary =
2 Dekker products per pass) accumulate the 2^20-scaled h into PSUM rows
0:8 + 32:40; a 4-op DVE epilogue folds the bias (per-partition
tensor_scalar add), sums hi+lo, and emits val = [h>=T]+[h>-T] as fp16.
mu = powers^T @ val runs as 2 small fp16 matmuls (3^7 = 2048+139 split
keeps the powers fp16-exact); an ordering-only dep (add_dep_helper)
anchors them behind the NEXT half's matmul stream so the in-order PE
queue never stalls on the vector epilogue.  Results stage through SBUF
(i32 convert on the scalar engine) and stream out incrementally on the
scalar DGE queue.

Steady state is DMA-bound: tensor ~87us busy, DMA ~102-113us stream,
~8us fill, ~13us tail+barriers => ~124us/core (fastest cores), up to
~137-147us on cores that lose chip-level HBM arbitration.
"""

import numpy as np

# exact fp32 threshold: minimal fp32 v with round(tanh(v)*SCALE) == 1
T_POS = float(np.uint32(0x3F0CCB15).view(np.float32))
SPLIT_SCALE = 1024.0  # 2^10 per operand; h is scaled by 2^20

N_CORES = 8
TOK_PER_CORE = 8192
D = 1280
K = 8
D_TILES = D // 128            # 10

MODE = "fp16x2t2"
N_PRODUCTS = 4                # hi*hi, hi*lo, lo*hi, lo*lo

# fp32-path tiling
SUPER = 4
TOKS = SUPER * 128            # 512
N_SUPER = TOK_PER_CORE // TOKS

# fp16x2-path tiling: 1024-token groups, matmul N=512 halves
GTOK = 1024
N_GROUP = TOK_PER_CORE // GTOK  # 8

_cached = {}


def _build_fp16x2(repeat=1):
    from contextlib import ExitStack

    from concourse import bacc, mybir, tile

    f16 = mybir.dt.float16
    f32 = mybir.dt.float32
    i32 = mybir.dt.int32

    nc = bacc.Bacc("TRN2", target_bir_lowering=False, debug=False)

    xhi_ap = nc.dram_tensor("xhi", [TOK_PER_CORE, D], f16, kind="ExternalInput").ap()
    xlo_ap = nc.dram_tensor("xlo", [TOK_PER_CORE, D], f16, kind="ExternalInput").ap()
    wthi_ap = nc.dram_tensor("wthi", [D, K], f16, kind="ExternalInput").ap()
    wtlo_ap = nc.dram_tensor("wtlo", [D, K], f16, kind="ExternalInput").ap()
    b_ap = nc.dram_tensor("bias", [1, K], f32, kind="ExternalInput").ap()
    pw_ap = nc.dram_tensor("powers", [K, 1], f32, kind="ExternalInput").ap()
    out_ap = nc.dram_tensor(
        "out", [1, TOK_PER_CORE], i32, kind="ExternalOutput"
    ).ap()

    T_HI = T_POS * SPLIT_SCALE * SPLIT_SCALE

    with tile.TileContext(nc) as tc, ExitStack() as ctx:
        const_pool = ctx.enter_context(tc.tile_pool(name="const", bufs=1))
        xt_pool = ctx.enter_context(tc.tile_pool(name="xt", bufs=4))
        val_pool = ctx.enter_context(tc.tile_pool(name="val", bufs=3))
        mu_pool = ctx.enter_context(tc.tile_pool(name="mu", bufs=1))
        ps_h = ctx.enter_context(tc.tile_pool(name="ps_h", bufs=4, space="PSUM"))
        ps_m = ctx.enter_context(tc.tile_pool(name="ps_m", bufs=2, space="PSUM"))

        # stacked stationary, 40 cols per d-tile: cols [0:8]=Whi_dt,
        # [32:40]=Wlo_dt (partition windows must start at multiples of 32;
        # the unused middle columns cost nothing — matmul time is N-bound)
        WP = 40
        wpair_sb = const_pool.tile([128, D_TILES * WP], f16)
        nc.vector.memset(wpair_sb[:], 0)
        nc.sync.dma_start(
            wpair_sb[:].rearrange("p (dt c) -> p dt c", dt=D_TILES)[:, :, 0:K],
            wthi_ap.rearrange("(dt p) k -> p dt k", p=128),
        )
        nc.sync.dma_start(
            wpair_sb[:].rearrange("p (dt c) -> p dt c", dt=D_TILES)[
                :, :, 32 : 32 + K
            ],
            wtlo_ap.rearrange("(dt p) k -> p dt k", p=128),
        )
        b_sb = const_pool.tile([1, K], f32)
        nc.sync.dma_start(b_sb[:], b_ap[:])
        pw_sb = const_pool.tile([K, 1], f32)
        nc.sync.dma_start(pw_sb[:], pw_ap[:])
        ones_row = const_pool.tile([1, 512], f32)
        nc.vector.memset(ones_row[:], 1.0)

        mu_i32 = mu_pool.tile([1, TOK_PER_CORE], i32)

        for _rep in range(repeat):
            for g in range(N_GROUP):
                t0 = g * GTOK
                # DMA-transpose loads: xthi[d, dt*GTOK + t] = xhi[t0+t, dt*128+d]
                xthi = xt_pool.tile([128, D_TILES * GTOK], f16, name="xthi")
                xtlo = xt_pool.tile([128, D_TILES * GTOK], f16, name="xtlo")
                for dt in range(D_TILES):
                    nc.sync.dma_start(
                        xthi[:, dt * GTOK : (dt + 1) * GTOK],
                        xhi_ap[t0 : t0 + GTOK, dt * 128 : (dt + 1) * 128],
                        transpose=True,
                    )
                    nc.sync.dma_start(
                        xtlo[:, dt * GTOK : (dt + 1) * GTOK],
                        xlo_ap[t0 : t0 + GTOK, dt * 128 : (dt + 1) * 128],
                        transpose=True,
                    )
                for half in range(2):
                    sl = lambda dt: slice(
                        dt * GTOK + half * 512, dt * GTOK + half * 512 + 512
                    )
                    # h40 rows 0-7 += Whi^T@(xthi+xtlo); rows 32-39 += Wlo^T@(...)
                    # all 4 Dekker products in 2 matmuls per d-tile
                    h40 = ps_h.tile([WP, 512], f32)
                    first = True
                    for dt in range(D_TILES):
                        for xsb in (xthi, xtlo):
                            nc.tensor.matmul(
                                h40[:],
                                lhsT=wpair_sb[:, dt * WP : (dt + 1) * WP],
                                rhs=xsb[:, sl(dt)],
                                start=first,
                                stop=False,
                            )
                            first = False
                    nc.tensor.matmul(
                        h40[0:K, :],
                        lhsT=b_sb[:],
                        rhs=ones_row[:],
                        start=False,
                        stop=True,
                    )

                    # h = rows[0:8] + rows[32:40]; val = [h >= T] + [h > -T]
                    # (tensor_tensor may read only one PSUM operand)
                    hlo_sb = val_pool.tile([K, 512], f32, name="hlo_sb")
                    nc.vector.tensor_copy(hlo_sb[:], h40[32 : 32 + K, :])
                    hsum = val_pool.tile([K, 512], f32, name="hsum")
                    nc.vector.tensor_add(hsum[:], h40[0:K, :], hlo_sb[:])
                    val1 = val_pool.tile([K, 512], f32, name="val1")
                    nc.vector.tensor_scalar(
                        out=val1[:],
                        in0=hsum[:],
                        scalar1=T_HI,
                        scalar2=None,
                        op0=mybir.AluOpType.is_ge,
                    )
                    val = val_pool.tile([K, 512], f32, name="val")
                    nc.vector.scalar_tensor_tensor(
                        out=val[:],
                        in0=hsum[:],
                        scalar=-T_HI,
                        in1=val1[:],
                        op0=mybir.AluOpType.is_gt,
                        op1=mybir.AluOpType.add,
                    )
                    # mu = powers^T @ val   (K=8 contraction)
                    mu_ps = ps_m.tile([1, 512], f32)
                    nc.tensor.matmul(
                        mu_ps[:], lhsT=pw_sb[:], rhs=val[:], start=True, stop=True
                    )
                    base = t0 + half * 512
                    nc.vector.tensor_copy(
                        mu_i32[:, base : base + 512], mu_ps[:]
                    )

        nc.sync.dma_start(out_ap[:], mu_i32[:])

    nc.compile()
    return nc


def _build_fp16x2t(repeat=1):
    """Host-pretransposed fp16x2: x is uploaded as one [128, G*DT*2*GTOK]
    fp16 tensor laid out [p, g, dt, hl, t] so each 1024-token group is a
    single fully-contiguous 5.2MB DMA (40KB/partition row).  No XBAR
    transpose, no per-tile DMA — the load runs at full HBM bandwidth and
    fans out across all 16 HW DGE engines automatically."""
    from contextlib import ExitStack

    from concourse import bacc, mybir, tile

    f16 = mybir.dt.float16
    f32 = mybir.dt.float32
    i32 = mybir.dt.int32

    nc = bacc.Bacc("TRN2", target_bir_lowering=False, debug=False)

    GFREE = D_TILES * 2 * GTOK          # 20480 per group
    xt_ap = nc.dram_tensor(
        "xt", [128, N_GROUP * GFREE], f16, kind="ExternalInput"
    ).ap()
    wthi_ap = nc.dram_tensor("wthi", [D, K], f16, kind="ExternalInput").ap()
    wtlo_ap = nc.dram_tensor("wtlo", [D, K], f16, kind="ExternalInput").ap()
    b_ap = nc.dram_tensor("bias", [1, K], f32, kind="ExternalInput").ap()
    pw_ap = nc.dram_tensor("powers", [K, 1], f32, kind="ExternalInput").ap()
    out_ap = nc.dram_tensor(
        "out", [1, TOK_PER_CORE], i32, kind="ExternalOutput"
    ).ap()

    T_HI = T_POS * SPLIT_SCALE * SPLIT_SCALE

    with tile.TileContext(nc) as tc, ExitStack() as ctx:
        const_pool = ctx.enter_context(tc.tile_pool(name="const", bufs=1))
        xt_pool = ctx.enter_context(tc.tile_pool(name="xt", bufs=4))
        val_pool = ctx.enter_context(tc.tile_pool(name="val", bufs=3))
        mu_pool = ctx.enter_context(tc.tile_pool(name="mu", bufs=1))
        ps_h = ctx.enter_context(tc.tile_pool(name="ps_h", bufs=4, space="PSUM"))
        ps_m = ctx.enter_context(tc.tile_pool(name="ps_m", bufs=2, space="PSUM"))

        # stacked stationary, 40 cols per d-tile: cols [0:8]=Whi_dt,
        # [32:40]=Wlo_dt (partition windows must start at multiples of 32)
        WP = 40
        wpair_sb = const_pool.tile([128, D_TILES * WP], f16)
        nc.vector.memset(wpair_sb[:], 0)
        nc.sync.dma_start(
            wpair_sb[:].rearrange("p (dt c) -> p dt c", dt=D_TILES)[:, :, 0:K],
            wthi_ap.rearrange("(dt p) k -> p dt k", p=128),
        )
        nc.sync.dma_start(
            wpair_sb[:].rearrange("p (dt c) -> p dt c", dt=D_TILES)[
                :, :, 32 : 32 + K
            ],
            wtlo_ap.rearrange("(dt p) k -> p dt k", p=128),
        )
        b_sb = const_pool.tile([1, K], f32)
        nc.sync.dma_start(b_sb[:], b_ap[:])
        pw_sb = const_pool.tile([K, 1], f32)
        nc.sync.dma_start(pw_sb[:], pw_ap[:])
        ones_row = const_pool.tile([1, 512], f32)
        nc.vector.memset(ones_row[:], 1.0)

        mu_i32 = mu_pool.tile([1, TOK_PER_CORE], i32)

        for _rep in range(repeat):
            for g in range(N_GROUP):
                xt = xt_pool.tile([128, GFREE], f16, name="xt")
                nc.sync.dma_start(
                    xt[:], xt_ap[:, g * GFREE : (g + 1) * GFREE]
                )
                for half in range(2):
                    # h40 rows 0-7 += Whi^T@(xhi+xlo); rows 32-39 += Wlo^T@(..)
                    h40 = ps_h.tile([WP, 512], f32)
                    first = True
                    for dt in range(D_TILES):
                        for hl in range(2):
                            base = (dt * 2 + hl) * GTOK + half * 512
                            nc.tensor.matmul(
                                h40[:],
                                lhsT=wpair_sb[:, dt * WP : (dt + 1) * WP],
                                rhs=xt[:, base : base + 512],
                                start=first,
                                stop=False,
                            )
                            first = False
                    nc.tensor.matmul(
                        h40[0:K, :],
                        lhsT=b_sb[:],
                        rhs=ones_row[:],
                        start=False,
                        stop=True,
                    )

                    # h = rows[0:8] + rows[32:40]; val = [h >= T] + [h > -T]
                    hlo_sb = val_pool.tile([K, 512], f32, name="hlo_sb")
                    nc.vector.tensor_copy(hlo_sb[:], h40[32 : 32 + K, :])
                    hsum = val_pool.tile([K, 512], f32, name="hsum")
                    nc.vector.tensor_add(hsum[:], h40[0:K, :], hlo_sb[:])
                    val1 = val_pool.tile([K, 512], f32, name="val1")
                    nc.vector.tensor_scalar(
                        out=val1[:],
                        in0=hsum[:],
                        scalar1=T_HI,
                        scalar2=None,
                        op0=mybir.AluOpType.is_ge,
                    )
                    val = val_pool.tile([K, 512], f32, name="val")
                    nc.vector.scalar_tensor_tensor(
                        out=val[:],
                        in0=hsum[:],
                        scalar=-T_HI,
                        in1=val1[:],
                        op0=mybir.AluOpType.is_gt,
                        op1=mybir.AluOpType.add,
                    )
                    # mu = powers^T @ val   (K=8 contraction)
                    mu_ps = ps_m.tile([1, 512], f32)
                    nc.tensor.matmul(
                        mu_ps[:], lhsT=pw_sb[:], rhs=val[:], start=True, stop=True
                    )
                    tbase = g * GTOK + half * 512
                    nc.vector.tensor_copy(
                        mu_i32[:, tbase : tbase + 512], mu_ps[:]
                    )

        nc.sync.dma_start(out_ap[:], mu_i32[:])

    nc.compile()
    return nc


def _build_fp16x2t2(repeat=1):
    """v3: host-pretransposed fp16x2 with software-pipelined mu matmuls.

    Host layout [p, g, half, dt, hl, t512]: each 512-token half is one
    contiguous 2.6MB DMA (20KB/partition row).  Per group, both halves'
    GEMMs accumulate into one [40,1024] PSUM tile (2 banks); the vector
    epilogue runs 1024-wide; the tiny mu matmul for group g is emitted
    after group g+1's main matmuls so the tensor queue never waits on
    the vector engine.  PSUM->SBUF copies ride the idle scalar engine."""
    from contextlib import ExitStack

    from concourse import bacc, mybir, tile

    f16 = mybir.dt.float16
    f32 = mybir.dt.float32
    i32 = mybir.dt.int32

    nc = bacc.Bacc("TRN2", target_bir_lowering=False, debug=False)

    HFREE = D_TILES * 2 * 512           # 10240 per half
    xt_ap = nc.dram_tensor(
        "xt", [128, N_GROUP * 2 * HFREE], f16, kind="ExternalInput"
    ).ap()
    WP = 40
    wpair_ap = nc.dram_tensor(
        "wpair", [128, D_TILES * WP], f16, kind="ExternalInput"
    ).ap()
    b2_ap = nc.dram_tensor("b2", [K, 1], f32, kind="ExternalInput").ap()
    # 3^k split into two fp16-exact columns (3^7=2187 = 2048+139)
    pwa_ap = nc.dram_tensor("pwa", [K, 1], f16, kind="ExternalInput").ap()
    pwb_ap = nc.dram_tensor("pwb", [K, 1], f16, kind="ExternalInput").ap()
    out_ap = nc.dram_tensor(
        "out", [1, TOK_PER_CORE], i32, kind="ExternalOutput"
    ).ap()

    T_HI = T_POS * SPLIT_SCALE * SPLIT_SCALE

    with tile.TileContext(nc) as tc, ExitStack() as ctx:
        const_pool = ctx.enter_context(tc.tile_pool(name="const", bufs=1))
        xt_pool = ctx.enter_context(tc.tile_pool(name="xt", bufs=4))
        val_pool = ctx.enter_context(tc.tile_pool(name="val", bufs=2))
        mu_pool = ctx.enter_context(tc.tile_pool(name="mu", bufs=2))
        ps_h = ctx.enter_context(tc.tile_pool(name="ps_h", bufs=2, space="PSUM"))
        ps_m = ctx.enter_context(tc.tile_pool(name="ps_m", bufs=2, space="PSUM"))

        from concourse.tile_rust import add_dep_helper

        # wpair is the first sync dispatch (every matmul needs it; the
        # scalar queue opens with a slow ACT table load); the other tiny
        # consts ride the scalar HWDGE queue; group 0's first half is
        # split into per-dt-pair sub-DMAs so matmuls start on dt0 while
        # the rest streams in
        wpair_sb = const_pool.tile([128, D_TILES * WP], f16)
        nc.sync.dma_start(wpair_sb[:], wpair_ap[:])
        b2_sb = const_pool.tile([K, 1], f32)
        nc.scalar.dma_start(b2_sb[:], b2_ap[:])
        pwa_sb = const_pool.tile([K, 1], f16)
        nc.scalar.dma_start(pwa_sb[:], pwa_ap[:])
        pwb_sb = const_pool.tile([K, 1], f16)
        nc.scalar.dma_start(pwb_sb[:], pwb_ap[:])

        xh_tiles = {}

        def issue_xh(g, half, pieces=1):
            t = xt_pool.tile([128, HFREE], f16, name=f"xh{half}")
            off = (g * 2 + half) * HFREE
            step = HFREE // pieces
            for i in range(pieces):
                nc.sync.dma_start(
                    t[:, i * step : (i + 1) * step],
                    xt_ap[:, off + i * step : off + (i + 1) * step],
                )
            xh_tiles[(g, half)] = t

        issue_xh(0, 0, pieces=5)
        issue_xh(0, 1, pieces=4)

        def emit_mu(val_t, g, half, anchor=None):
            mu_ps = ps_m.tile([1, 512], f32, name=f"mu{half}")
            mm1 = nc.tensor.matmul(
                mu_ps[:], lhsT=pwa_sb[:], rhs=val_t[:], start=True, stop=False
            )
            if anchor is not None:
                # ordering-only edge: keep the scheduler from hoisting the
                # mu matmul ahead of the next half's stream (it would stall
                # the in-order tensor queue waiting on the vector epilogue)
                add_dep_helper(
                    mm1.ins, anchor.ins, sync=False, reason="delay mu"
                )
            nc.tensor.matmul(
                mu_ps[:], lhsT=pwb_sb[:], rhs=val_t[:], start=False, stop=True
            )
            # stage in SBUF (i32 convert) then stream out on the scalar
            # DGE queue; the copy and DMA stay on one queue so no extra
            # cross-engine sems land on the tail
            stage = mu_pool.tile([1, 512], i32, name=f"stage{half}")
            nc.scalar.copy(stage[:], mu_ps[:])
            tbase = g * GTOK + half * 512
            nc.scalar.dma_start(out_ap[:, tbase : tbase + 512], stage[:])

        for _rep in range(repeat):
            pending = []
            for g in range(N_GROUP):
                if g + 1 < N_GROUP:
                    issue_xh(g + 1, 0, pieces=4)
                    issue_xh(g + 1, 1, pieces=4)
                for half in range(2):
                    xh = xh_tiles.pop((g, half))
                    h40 = ps_h.tile([WP, 512], f32, name=f"h{half}")
                    first = True
                    anchor = None
                    for dt in range(D_TILES):
                        for hl in range(2):
                            base = (dt * 2 + hl) * 512
                            mm = nc.tensor.matmul(
                                h40[:],
                                lhsT=wpair_sb[:, dt * WP : (dt + 1) * WP],
                                rhs=xh[:, base : base + 512],
                                start=first,
                                stop=(dt == D_TILES - 1 and hl == 1),
                            )
                            first = False
                            if dt == 7 and hl == 0:
                                anchor = mm
                    # tensor queue: flush mu of the previous half (its
                    # epilogue ran during this half's matmuls)
                    while pending:
                        emit_mu(*pending.pop(0), anchor=anchor)

                    # epilogue: h = rows[0:8]+(rows[32:40]+b);
                    # val = [h >= T] + [h > -T]   (fp16, for fast mu matmul)
                    hlo_b = val_pool.tile([K, 512], f32, name=f"hlo_b{half}")
                    nc.vector.tensor_scalar(
                        out=hlo_b[:],
                        in0=h40[32 : 32 + K, :],
                        scalar1=b2_sb[:],
                        scalar2=None,
                        op0=mybir.AluOpType.add,
                    )
                    hsum = val_pool.tile([K, 512], f32, name=f"hsum{half}")
                    nc.vector.tensor_add(hsum[:], h40[0:K, :], hlo_b[:])
                    val1 = val_pool.tile([K, 512], f32, name=f"val1{half}")
                    nc.vector.tensor_scalar(
                        out=val1[:],
                        in0=hsum[:],
                        scalar1=T_HI,
                        scalar2=None,
                        op0=mybir.AluOpType.is_ge,
                    )
                    val = val_pool.tile([K, 512], f16, name=f"val{half}")
                    nc.vector.scalar_tensor_tensor(
                        out=val[:],
                        in0=hsum[:],
                        scalar=-T_HI,
                        in1=val1[:],
                        op0=mybir.AluOpType.is_gt,
                        op1=mybir.AluOpType.add,
                    )
                    pending.append((val, g, half))
            while pending:
                emit_mu(*pending.pop(0))

    nc.compile()
    return nc


def _build_fp32(repeat=1, stages="full"):
    from contextlib import ExitStack

    from concourse import bacc, masks, mybir, tile

    do_t = stages in ("full", "dma_t")
    do_mm = stages in ("full", "dma_mm")

    f32 = mybir.dt.float32
    i32 = mybir.dt.int32

    nc = bacc.Bacc("TRN2", target_bir_lowering=False, debug=False)

    x_ap = nc.dram_tensor("x", [TOK_PER_CORE, D], f32, kind="ExternalInput").ap()
    wt_ap = nc.dram_tensor("wt", [D, K], f32, kind="ExternalInput").ap()
    b_ap = nc.dram_tensor("bias", [1, K], f32, kind="ExternalInput").ap()
    pw_ap = nc.dram_tensor("powers", [K, 1], f32, kind="ExternalInput").ap()
    out_ap = nc.dram_tensor(
        "out", [1, TOK_PER_CORE], i32, kind="ExternalOutput"
    ).ap()
    probe_ap = None
    if stages != "full":
        probe_ap = nc.dram_tensor(
            "probe_out", [128, 8], f32, kind="ExternalOutput"
        ).ap()

    with tile.TileContext(nc) as tc, ExitStack() as ctx:
        const_pool = ctx.enter_context(tc.tile_pool(name="const", bufs=1))
        x_pool = ctx.enter_context(tc.tile_pool(name="x", bufs=4))
        xt_pool = ctx.enter_context(tc.tile_pool(name="xt", bufs=4))
        val_pool = ctx.enter_context(tc.tile_pool(name="val", bufs=3))
        mu_pool = ctx.enter_context(tc.tile_pool(name="mu", bufs=1))
        ps_t = ctx.enter_context(tc.tile_pool(name="ps_t", bufs=5, space="PSUM"))
        ps_h = ctx.enter_context(tc.tile_pool(name="ps_h", bufs=2, space="PSUM"))
        ps_m = ctx.enter_context(tc.tile_pool(name="ps_m", bufs=1, space="PSUM"))

        identity = const_pool.tile([128, 128], f32)
        masks.make_identity(nc, identity[:])

        wt_sb = const_pool.tile([128, D_TILES * K], f32)
        nc.sync.dma_start(
            wt_sb[:].rearrange("p (dt k) -> p dt k", dt=D_TILES),
            wt_ap.rearrange("(dt p) k -> p dt k", p=128),
        )
        b_sb = const_pool.tile([1, K], f32)
        nc.sync.dma_start(b_sb[:], b_ap[:])
        pw_sb = const_pool.tile([K, 1], f32)
        nc.sync.dma_start(pw_sb[:], pw_ap[:])
        ones_row = const_pool.tile([1, TOKS], f32)
        nc.vector.memset(ones_row[:], 1.0)

        mu_i32 = mu_pool.tile([1, TOK_PER_CORE], i32)
        nc.vector.memset(mu_i32[:], 0)

        probe = None
        if stages != "full":
            probe = mu_pool.tile([128, 8], f32, name="probe")

        xt_const = None
        if stages == "dma_mm":
            xt_const = const_pool.tile([128, D_TILES * TOKS], f32)
            nc.vector.memset(xt_const[:], 0.25)

        for _rep in range(repeat):
            for st in range(N_SUPER):
                xs = x_pool.tile([128, SUPER * D], f32)
                nc.sync.dma_start(
                    xs[:].rearrange("p (j d) -> p j d", j=SUPER),
                    x_ap.rearrange("(s j p) d -> s p j d", j=SUPER, p=128)[st],
                )
                xt = None
                if do_t:
                    xt = xt_pool.tile([128, D_TILES * TOKS], f32)
                    ci = 0
                    for j in range(SUPER):
                        for dt in range(D_TILES):
                            ps = ps_t.tile([128, 128], f32)
                            nc.tensor.transpose(
                                ps[:],
                                xs[:, j * D + dt * 128 : j * D + (dt + 1) * 128],
                                identity[:],
                            )
                            dst = xt[
                                :, dt * TOKS + j * 128 : dt * TOKS + (j + 1) * 128
                            ]
                            if ci % 9 < 4:
                                nc.scalar.copy(dst, ps[:])
                            else:
                                nc.vector.tensor_copy(dst, ps[:])
                            ci += 1
                if stages == "dma_mm":
                    xt = xt_const
                    nc.vector.tensor_copy(probe[:], xs[:, 0:8])
                if not do_mm:
                    src = xt if do_t else xs
                    nc.vector.tensor_copy(probe[:], src[:, 0:8])
                    continue

                h = ps_h.tile([K, TOKS], f32)
                for dt in range(D_TILES):
                    nc.tensor.matmul(
                        h[:],
                        lhsT=wt_sb[:, dt * K : (dt + 1) * K],
                        rhs=xt[:, dt * TOKS : (dt + 1) * TOKS],
                        start=(dt == 0),
                        stop=False,
                    )
                nc.tensor.matmul(
                    h[:], lhsT=b_sb[:], rhs=ones_row[:], start=False, stop=True
                )

                val1 = val_pool.tile([K, TOKS], f32)
                nc.vector.tensor_scalar(
                    out=val1[:],
                    in0=h[:],
                    scalar1=T_POS,
                    scalar2=None,
                    op0=mybir.AluOpType.is_ge,
                )
                val = val_pool.tile([K, TOKS], f32)
                nc.vector.scalar_tensor_tensor(
                    out=val[:],
                    in0=h[:],
                    scalar=-T_POS,
                    in1=val1[:],
                    op0=mybir.AluOpType.is_gt,
                    op1=mybir.AluOpType.add,
                )
                mu_ps = ps_m.tile([1, TOKS], f32)
                nc.tensor.matmul(
                    mu_ps[:], lhsT=pw_sb[:], rhs=val[:], start=True, stop=True
                )
                nc.vector.tensor_copy(
                    mu_i32[:, st * TOKS : (st + 1) * TOKS], mu_ps[:]
                )

        nc.sync.dma_start(out_ap[:], mu_i32[:])
        if probe_ap is not None:
            nc.sync.dma_start(probe_ap[:], probe[:])

    nc.compile()
    return nc


def _build_program(repeat=1, stages="full", mode=None):
    mode = mode or MODE
    if mode == "fp16x2t2":
        assert stages == "full"
        return _build_fp16x2t2(repeat)
    if mode == "fp16x2t":
        assert stages == "full"
        return _build_fp16x2t(repeat)
    if mode == "fp16x2":
        assert stages == "full"
        return _build_fp16x2(repeat)
    return _build_fp32(repeat, stages)


def _get_program(repeat=1, mode=None):
    key = ("nc", repeat, mode or MODE)
    if key not in _cached:
        _cached[key] = _build_program(repeat, mode=mode)
    return _cached[key]


def _split_f16(a32):
    hi = a32.astype(np.float16)
    lo = (a32 - hi.astype(np.float32)).astype(np.float16)
    return hi, lo


def make_in_maps(x, W, b, mode=None):
    mode = mode or MODE
    xf = np.ascontiguousarray(x.reshape(-1, D), dtype=np.float32)
    b1 = np.ascontiguousarray(b.reshape(1, K), dtype=np.float32)
    powers = (3.0 ** np.arange(K, dtype=np.float32)).reshape(K, 1).astype(np.float32)
    if mode in ("fp16x2t", "fp16x2t2"):
        xs = xf * np.float32(SPLIT_SCALE)
        xhi, xlo = _split_f16(xs)
        ws = np.ascontiguousarray(W.T, dtype=np.float32) * np.float32(SPLIT_SCALE)
        wthi, wtlo = _split_f16(ws)
        bs = b1 * np.float32(SPLIT_SCALE * SPLIT_SCALE)
        if mode == "fp16x2t":
            # [core*g*t, dt*p] -> [core][p, g, dt, hl, t] contiguous
            r = lambda a: a.reshape(N_CORES, N_GROUP, GTOK, D_TILES, 128)
            pair = np.stack([r(xhi), r(xlo)], axis=3)  # [c, g, t, hl, dt, p]
            pair = np.ascontiguousarray(pair.transpose(0, 5, 1, 4, 3, 2))
            pair = pair.reshape(N_CORES, 128, N_GROUP * D_TILES * 2 * GTOK)
            return [
                {
                    "xt": pair[c],
                    "wthi": wthi,
                    "wtlo": wtlo,
                    "bias": bs,
                    "powers": powers,
                }
                for c in range(N_CORES)
            ]
        # fp16x2t2: [core][p, g, half, dt, hl, t512] contiguous
        r = lambda a: a.reshape(N_CORES, N_GROUP, 2, 512, D_TILES, 128)
        pair = np.stack([r(xhi), r(xlo)], axis=4)  # [c,g,half,t,hl,dt,p]
        pair = np.ascontiguousarray(pair.transpose(0, 6, 1, 2, 5, 4, 3))
        pair = pair.reshape(N_CORES, 128, N_GROUP * D_TILES * 2 * GTOK)
        # prepacked stationary: per d-tile 40 cols, [0:8]=Whi, [32:40]=Wlo
        WP = 40
        wpair = np.zeros((128, D_TILES * WP), dtype=np.float16)
        wr = lambda a: a.reshape(D_TILES, 128, K).transpose(1, 0, 2)
        wpair.reshape(128, D_TILES, WP)[:, :, 0:K] = wr(wthi)
        wpair.reshape(128, D_TILES, WP)[:, :, 32 : 32 + K] = wr(wtlo)
        b2 = np.ascontiguousarray(bs.reshape(K, 1))
        pwa = np.array([1, 3, 9, 27, 81, 243, 729, 2048], dtype=np.float16)
        pwb = np.array([0, 0, 0, 0, 0, 0, 0, 139], dtype=np.float16)
        return [
            {
                "xt": pair[c],
                "wpair": wpair,
                "b2": b2,
                "pwa": pwa.reshape(K, 1),
                "pwb": pwb.reshape(K, 1),
            }
            for c in range(N_CORES)
        ]
    if mode == "fp16x2":
        xs = xf * np.float32(SPLIT_SCALE)
        xhi, xlo = _split_f16(xs)
        ws = np.ascontiguousarray(W.T, dtype=np.float32) * np.float32(SPLIT_SCALE)
        wthi, wtlo = _split_f16(ws)
        bs = b1 * np.float32(SPLIT_SCALE * SPLIT_SCALE)
        return [
            {
                "xhi": xhi[c * TOK_PER_CORE : (c + 1) * TOK_PER_CORE],
                "xlo": xlo[c * TOK_PER_CORE : (c + 1) * TOK_PER_CORE],
                "wthi": wthi,
                "wtlo": wtlo,
                "bias": bs,
                "powers": powers,
            }
            for c in range(N_CORES)
        ]
    wt = np.ascontiguousarray(W.T, dtype=np.float32)
    return [
        {
            "x": xf[c * TOK_PER_CORE : (c + 1) * TOK_PER_CORE],
            "wt": wt,
            "bias": b1,
            "powers": powers,
        }
        for c in range(N_CORES)
    ]


def kernel(x: np.ndarray, W: np.ndarray, b: np.ndarray) -> np.ndarray:
    from concourse.bass_utils import run_bass_kernel_spmd

    nc = _get_program()

    B, T, Dx = x.shape
    assert (B * T, Dx) == (N_CORES * TOK_PER_CORE, D)
    in_maps = make_in_maps(x, W, b)
    res = run_bass_kernel_spmd(nc, in_maps, list(range(N_CORES)))
    mu = np.concatenate(
        [res.results[c]["out"].reshape(-1) for c in range(N_CORES)]
    )
    return mu.reshape(B, T).astype(np.int32)

